# revision 34
# baseline (speedup 1.0000x reference)
"""MoE expert-collection grouped GEMM for Trainium2, expert-parallel over 8
NeuronCores, fp8 DoubleRow matmuls, weight-stationary / transposed output.

Problem (hardcoded shapes):
  sorted_features  [65536, 1024] f32   tokens sorted by expert, 4096/expert
  expert_ids_sorted[65536] i32         unused: split is static equal-count
  routing_matrix   [1024, 2048, 16] f32
  routing_bias     [2048, 16] f32
  out = silu(x_e @ W_e + b_e) per expert  -> [65536, 2048] f32

Sharding: expert-parallel, 2 experts (= 8192 contiguous sorted tokens) per
core.

Design (weight-stationary): each matmul computes a [128 outs, 512 toks]
PSUM tile: lhsT = w chunk [128, 2(DR), 128 outs], rhs = xT chunk
[128, 2(DR), 512 toks], accumulated over 4 k-pairs.  The output is produced
TRANSPOSED (yt [2048, 8192] f16) and de-transposed on the host.  This makes
the bias per-PARTITION, so the whole PSUM drain is ONE scalar-engine
activation: silu(psum * OUT_SCALE + bias_fp32) reading PSUM directly --
no DVE work at all.  Tokens are processed in stripe-PAIR blocks (1024
tokens) so one [128, 2, 512] two-bank ACT drains a whole ob, keeping the
scalar engine under ~80% busy; y stores ride the sync ring (the scalar
ring is ACT-only) at 4-ob granularity (2KB DRAM runs).

Head shaping: stripe 0 runs first against only out-blocks 0-7 so the
critical preload is half the expert's weights; out-blocks 8-15 of stripe 0
run as a third block against the still-resident x.  Head DMAs are few and
large (2-4KB per-partition lines) because walrus shares completion
semaphores across queues -- many small head DMAs serialize on sem reuse.
6 zero-matmul warmups flip the PE HAM clock-gate while the preload
streams.  The final block's last out-blocks drain per-stripe with small
sync-ring stores to shorten the tail.
"""

import numpy as np
import ml_dtypes

import concourse.bass as bass
import concourse.mybir as mybir
import concourse.tile as tile
from concourse.bass_utils import run_bass_kernel_spmd

N_CORES = 8
N_TOKENS = 65536
D_IN = 1024
D_OUT = 2048
N_EXPERTS = 16
E_PER_CORE = N_EXPERTS // N_CORES        # 2
TOK_PER_CORE = N_TOKENS // N_CORES       # 8192
TOK_PER_EXPERT = N_TOKENS // N_EXPERTS   # 4096

P = 128
KP = 4                     # DoubleRow k-pairs (256 contraction each)
TS = 512                   # token stripe (matmul moving free dim)
N_STRIPES = TOK_PER_CORE // TS           # 16
OBW = 128                  # out-feature block (psum partition dim)
N_OB = D_OUT // OBW        # 16

S_X = 4.0                  # keeps x (std 1) in e4m3 normal range
S_W = 128.0                # keeps W (std ~0.0054) out of e4m3 subnormals
OUT_SCALE = 1.0 / (S_X * S_W)

N_WARMUP_MM = 8

F32 = mybir.dt.float32
F16 = mybir.dt.float16
F8 = mybir.dt.float8e4
NP_F8 = ml_dtypes.float8_e4m3

DR = mybir.MatmulPerfMode.DoubleRow
SILU = mybir.ActivationFunctionType.Silu


def _split_multi_waits(nc):
    """This container's walrus encodes at most ONE sync-wait per instruction;
    hoist extras onto single-wait NoOps inserted just before, same engine."""
    for fn in nc.m.functions:
        for bb in fn.blocks:
            insts = list(bb.instructions)
            out = []
            dirty = False
            for inst in insts:
                si = inst.sync_info
                waits = list(si.on_wait) if si and si.on_wait else []
                if len(waits) > 1:
                    dirty = True
                    for j, w in enumerate(waits[:-1]):
                        nop = mybir.InstNoOp(
                            name=f"{inst.name}-prewait{j}", ins=[], outs=[]
                        )
                        nop.engine = inst.engine
                        nop.sync_info = mybir.SyncInfo(on_wait=[w], on_update=[])
                        out.append(nop)
                    inst.sync_info = mybir.SyncInfo(
                        on_wait=[waits[-1]],
                        on_update=list(si.on_update) if si.on_update else [],
                    )
                out.append(inst)
            if dirty:
                bb.instructions = out


def build_kernel():
    nc = bass.Bass()
    # xt[s, kp, p, j*TS+t] = S_X * X[s*TS+t, kp*256 + j*128 + p]
    xt = nc.dram_tensor("xt", [N_STRIPES, KP, P, 2 * TS], F8,
                        kind="ExternalInput")
    # w[e, ob, p, kp*256 + j*128 + i] = S_W * W_e[kp*256 + j*128 + p, ob*128+i]
    w = nc.dram_tensor("w", [E_PER_CORE, N_OB, P, KP * 2 * OBW], F8,
                       kind="ExternalInput")
    # bb[e, p, ob] = bias[ob*128 + p] (exact fp32, applied inside ACT)
    bb = nc.dram_tensor("bb", [E_PER_CORE, P, N_OB], F32, kind="ExternalInput")
    # transposed output; host does yt.T
    yt = nc.dram_tensor("yt", [D_OUT, TOK_PER_CORE], F16, kind="ExternalOutput")

    # block schedule: (expert, [stripe ids], ob_lo, ob_hi)
    blocks = [
        (0, [0], 0, 8),         # head: small critical preload
        (0, [1], 0, 16),
        (0, [0], 8, 16),        # finish stripe 0 against resident x
        (0, [2, 3], 0, 16),
        (0, [4, 5], 0, 16),
        (0, [6, 7], 0, 16),
        (1, [8, 9], 0, 16),
        (1, [10, 11], 0, 16),
        (1, [12, 13], 0, 16),
        (1, [14, 15], 0, 16),   # tail pair; last obs drain per-stripe
    ]

    with tile.TileContext(nc) as tc:
        with (
            tc.tile_pool(name="persist", bufs=1) as persist,
            tc.tile_pool(name="xs", bufs=5) as xsp,
            tc.tile_pool(name="outs", bufs=4) as outs,
            tc.tile_pool(name="psum", bufs=3, space="PSUM") as psump,
        ):
            # --- PE warm-up: matmuls over zeroed scratch, no DMA deps.
            # Sized to keep the PE busy from ~8us until the first real
            # matmul's operands land (~11.5us): an idle PE never reaches
            # the HAM 8/8 clock state and the whole first expert would run
            # at 1.2GHz.
            # N=512 warmups accumulate ~4.3us of PE-busy while the preload
            # streams, flipping the HAM to 8/8 just before the real stream
            # starts.
            zs = persist.tile([P, 2, TS], F8, name="warm_src")
            nc.vector.memset(zs[:], 0.0)
            ps_warm = psump.tile([P, 2, TS], F32, tag="ps2", name="ps_warm")
            for i in range(N_WARMUP_MM):
                nc.tensor.matmul(
                    ps_warm[:, 0, :],
                    lhsT=zs[:, :, 0:P],
                    rhs=zs[:],
                    start=True, stop=True,
                    perf_mode=DR,
                    skip_group_check=True,
                )

            # --- persistent weight/bias tiles ---
            # e0: obs 0-7 as 2-ob tiles (2KB lines, fine head granularity),
            # obs 8-15 and all of e1 as 4-ob tiles (4KB lines).
            w8d = {q: persist.tile([P, 2, KP, 2, OBW], F8, name=f"w8d_{q}")
                   for q in range(4)}
            w8q = {}
            for e in range(E_PER_CORE):
                q0 = 2 if e == 0 else 0
                for q in range(q0, 4):
                    w8q[(e, q)] = persist.tile([P, 4, KP, 2, OBW], F8,
                                               name=f"w8q_{e}_{q}")
            b_sb = [persist.tile([P, N_OB], F32, name=f"bias_{e}")
                    for e in range(E_PER_CORE)]

            def w_ap(e, ob, kp):
                if e == 0 and ob < 8:
                    return w8d[ob // 2][:, ob % 2, kp, :, :]
                return w8q[(e, ob // 4)][:, ob % 4, kp, :, :]

            def load_w2(q, eng):
                eng.dma_start(
                    w8d[q][:],
                    w[0, 2 * q:2 * q + 2].rearrange(
                        "o p (k j i) -> p o k j i", k=KP, j=2))

            def load_w4(e, q, eng):
                eng.dma_start(
                    w8q[(e, q)][:],
                    w[e, 4 * q:4 * q + 4].rearrange(
                        "o p (k j i) -> p o k j i", k=KP, j=2))

            # x tiles: stripes 0/1 as half-stripe (2 k-pair) tiles for head
            # granularity; the rest as full-stripe tiles (4KB lines).
            xh = {}
            x_tiles = {}

            def load_xhalf(s, h, eng=None):
                xh[(s, h)] = xsp.tile([P, 2, 2, TS], F8, tag="xh",
                                      name=f"xh_{s}_{h}")
                (eng or nc.sync).dma_start(
                    xh[(s, h)][:],
                    xt[s, 2 * h:2 * h + 2].rearrange(
                        "k p (j t) -> p k j t", j=2))

            def load_stripe(s):
                x_tiles[s] = xsp.tile([P, KP, 2, TS], F8, tag="xs",
                                      name=f"xs_{s}")
                nc.sync.dma_start(
                    x_tiles[s][:],
                    xt[s].rearrange("k p (j t) -> p k j t", j=2))

            def x_ap(s, kp):
                if s in (0, 1):
                    return xh[(s, kp // 2)][:, kp % 2, :, :]
                return x_tiles[s][:, kp, :, :]

            # --- head preload, need-ordered, few+large DMAs ---
            # The first-matmul operands land in parallel: w obs0-3 (4KB
            # lines) leads the sync queue while x stripe-0 halves lead the
            # scalar queue.  Biases go on the gpsimd software queue so
            # their completion semaphores don't serialize later hardware-
            # queue loads (walrus rotates a small global semaphore pool).
            load_xhalf(0, 0)             # sync: stripe 0, kp 0-1
            load_w2(0, nc.scalar)        # scalar: obs 0-1
            load_xhalf(0, 1, nc.gpsimd)  # stripe 0 kp 2-3 via SWDGE (3rd
                                         # parallel path at the head)
            load_w2(1, nc.sync)          # obs 2-3
            load_w2(2, nc.scalar)        # obs 4-5
            load_w2(3, nc.scalar)        # obs 6-7
            nc.gpsimd.dma_start(b_sb[0][:], bb[0])
            # stripe-1 halves follow on sync; e0 upper weights wait until
            # blkA is underway so they don't contend with its loads
            load_xhalf(1, 0)
            load_xhalf(1, 1)
            nc.gpsimd.dma_start(b_sb[1][:], bb[1])

            # x/w prefetch emitted on sync at the start of block bi
            prefetch = {
                1: [lambda: load_w4(0, 2, nc.sync),
                    lambda: load_w4(0, 3, nc.sync),
                    lambda: load_stripe(2), lambda: load_stripe(3)],
                2: [lambda: load_stripe(4), lambda: load_stripe(5)],
                3: [lambda: load_stripe(6), lambda: load_stripe(7),
                    lambda: load_w4(1, 0, nc.sync),
                    lambda: load_w4(1, 1, nc.sync)],
                4: [lambda: load_stripe(8), lambda: load_stripe(9),
                    lambda: load_w4(1, 2, nc.sync),
                    lambda: load_w4(1, 3, nc.sync)],
                5: [lambda: load_stripe(10), lambda: load_stripe(11)],
                6: [lambda: load_stripe(12), lambda: load_stripe(13)],
                7: [lambda: load_stripe(14), lambda: load_stripe(15)],
            }

            n_blocks = len(blocks)
            for bi, (e, stripes, ob_lo, ob_hi) in enumerate(blocks):
                for fn in prefetch.get(bi, []):
                    fn()
                pair = len(stripes) == 2
                span = len(stripes) * TS
                t0 = stripes[0] * TS
                last_block = bi == n_blocks - 1
                for og in range(ob_lo, ob_hi, 4):
                    obs = list(range(og, min(og + 4, ob_hi)))
                    tail_og = last_block and og + 4 >= ob_hi
                    if not tail_og:
                        tag = "ytp" if pair else "yts"
                        y4 = outs.tile([P, 4, span], F16, tag=tag, name="y4")
                    if pair:
                        for oi, ob in enumerate(obs):
                            ps = psump.tile([P, 2, TS], F32, tag="ps2",
                                            name="ps2")
                            ps_of = [ps[:, 0, :], ps[:, 1, :]]
                            for kp in range(KP):
                                for si in range(2):
                                    nc.tensor.matmul(
                                        ps_of[si],
                                        lhsT=w_ap(e, ob, kp),
                                        rhs=x_ap(stripes[si], kp),
                                        start=(kp == 0),
                                        stop=(kp == KP - 1),
                                        perf_mode=DR,
                                    )
                            bias_ap = b_sb[e][:, ob:ob + 1]
                            if not tail_og:
                                # one ACT drains the whole ob (both banks)
                                nc.scalar.activation(
                                    y4[:, oi, :].rearrange(
                                        "p (s t) -> p s t", s=2),
                                    ps[:], SILU, bias=bias_ap,
                                    scale=OUT_SCALE)
                            elif ob < ob_hi - 2:
                                # tail obs 12-13: per-ob drain + store
                                y1 = outs.tile([P, 2, TS], F16, tag="ytm",
                                               name="y1")
                                nc.scalar.activation(y1[:], ps[:], SILU,
                                                     bias=bias_ap,
                                                     scale=OUT_SCALE)
                                nc.sync.dma_start(
                                    yt[ob * OBW:(ob + 1) * OBW,
                                       t0:t0 + span],
                                    y1[:])
                            else:
                                # final two obs: per-stripe drains + stores
                                # on the now-idle sync ring; the scalar
                                # ring stays ACT-only so the final drain
                                # chain is short
                                for si, s in enumerate(stripes):
                                    ys = outs.tile([P, TS], F16, tag="ytt",
                                                   name="ys")
                                    nc.scalar.activation(
                                        ys[:], ps_of[si], SILU,
                                        bias=bias_ap, scale=OUT_SCALE)
                                    nc.sync.dma_start(
                                        yt[ob * OBW:(ob + 1) * OBW,
                                           s * TS:(s + 1) * TS],
                                        ys[:])
                    else:
                        # stripe block: two obs share one 2-bank psum tile;
                        # each ob's ACT is emitted as soon as its bank's
                        # accumulation group closes
                        s = stripes[0]
                        for half in range(2):
                            ps = psump.tile([P, 2, TS], F32, tag="ps2",
                                            name="ps2")
                            for oi2 in range(2):
                                ob = og + 2 * half + oi2
                                for kp in range(KP):
                                    nc.tensor.matmul(
                                        ps[:, oi2, :],
                                        lhsT=w_ap(e, ob, kp),
                                        rhs=x_ap(s, kp),
                                        start=(kp == 0),
                                        stop=(kp == KP - 1),
                                        perf_mode=DR,
                                    )
                                nc.scalar.activation(
                                    y4[:, 2 * half + oi2, :],
                                    ps[:, oi2, :], SILU,
                                    bias=b_sb[e][:, ob:ob + 1],
                                    scale=OUT_SCALE)
                    if not tail_og:
                        dst = yt[og * OBW:(og + 4) * OBW,
                                 t0:t0 + span].rearrange(
                                     "(o p) t -> p o t", p=P)
                        nc.sync.dma_start(dst, y4[:])

    _split_multi_waits(nc)
    return nc


_NC_CACHE = None


def _get_nc():
    global _NC_CACHE
    if _NC_CACHE is None:
        _NC_CACHE = build_kernel()
    return _NC_CACHE


def _in_maps(sorted_features, routing_matrix, routing_bias):
    maps = []
    for c in range(N_CORES):
        rows = slice(c * TOK_PER_CORE, (c + 1) * TOK_PER_CORE)
        es = slice(c * E_PER_CORE, (c + 1) * E_PER_CORE)
        # [s, kp, p, j*TS+t] = S_X * X[s*TS+t, kp*256 + j*128 + p]
        xt_c = np.ascontiguousarray(
            (sorted_features[rows] * S_X)
            .astype(NP_F8)
            .reshape(N_STRIPES, TS, KP, 2, P)   # [s, t, kp, j, p]
            .transpose(0, 2, 4, 3, 1)           # [s, kp, p, j, t]
            .reshape(N_STRIPES, KP, P, 2 * TS)
        )
        # [e, ob, p, kp*256 + j*128 + i]
        w_c = np.ascontiguousarray(
            (routing_matrix[:, :, es] * S_W)
            .astype(NP_F8)
            .transpose(2, 0, 1)                      # [e, d_in, d_out]
            .reshape(E_PER_CORE, KP, 2, P, N_OB, OBW)  # [e,kp,j,p,ob,i]
            .transpose(0, 4, 3, 1, 2, 5)             # [e, ob, p, kp, j, i]
            .reshape(E_PER_CORE, N_OB, P, KP * 2 * OBW)
        )
        # [e, p, ob] = bias[ob*128 + p]  (exact fp32)
        b_c = np.ascontiguousarray(
            routing_bias[:, es].T                    # [e, d_out]
            .reshape(E_PER_CORE, N_OB, P)
            .transpose(0, 2, 1)
            .astype(np.float32)
        )
        maps.append({"xt": xt_c, "w": w_c, "bb": b_c})
    return maps


def run(sorted_features, routing_matrix, routing_bias, **run_kwargs):
    nc = _get_nc()
    maps = _in_maps(sorted_features, routing_matrix, routing_bias)
    res = run_bass_kernel_spmd(nc, maps, core_ids=list(range(N_CORES)),
                               **run_kwargs)
    out = np.empty((N_TOKENS, D_OUT), dtype=np.float32)
    for c in range(N_CORES):
        yt_c = np.asarray(res.results[c]["yt"])
        out[c * TOK_PER_CORE:(c + 1) * TOK_PER_CORE] = yt_c.T
    return out, res


def kernel(sorted_features, expert_ids_sorted, routing_matrix, routing_bias):
    assert sorted_features.shape == (N_TOKENS, D_IN)
    assert routing_matrix.shape == (D_IN, D_OUT, N_EXPERTS)
    assert routing_bias.shape == (D_OUT, N_EXPERTS)
    out, _ = run(
        np.asarray(sorted_features, dtype=np.float32),
        np.asarray(routing_matrix, dtype=np.float32),
        np.asarray(routing_bias, dtype=np.float32),
    )
    return out


# revision 35
# speedup vs baseline: 1.0535x; 1.0535x over previous
"""MoE expert-collection grouped GEMM for Trainium2, expert-parallel over 8
NeuronCores, fp8 DoubleRow matmuls, weight-stationary / transposed output.

Problem (hardcoded shapes):
  sorted_features  [65536, 1024] f32   tokens sorted by expert, 4096/expert
  expert_ids_sorted[65536] i32         unused: split is static equal-count
  routing_matrix   [1024, 2048, 16] f32
  routing_bias     [2048, 16] f32
  out = silu(x_e @ W_e + b_e) per expert  -> [65536, 2048] f32

Sharding: expert-parallel, 2 experts (= 8192 contiguous sorted tokens) per
core.

Design (weight-stationary): each matmul computes a [128 outs, 512 toks]
PSUM tile: lhsT = w chunk [128, 2(DR), 128 outs], rhs = xT chunk
[128, 2(DR), 512 toks], accumulated over 4 k-pairs.  The output is produced
TRANSPOSED (yt [2048, 8192] f16) and de-transposed on the host.  This makes
the bias per-PARTITION, so the whole PSUM drain is ONE scalar-engine
activation: silu(psum * OUT_SCALE + bias_fp32) reading PSUM directly --
no DVE work at all.  Tokens are processed in stripe-PAIR blocks (1024
tokens) so one [128, 2, 512] two-bank ACT drains a whole ob, keeping the
scalar engine under ~80% busy; y stores ride the sync ring (the scalar
ring is ACT-only) at 4-ob granularity (2KB DRAM runs).

Head shaping: stripe 0 runs first against only out-blocks 0-7 so the
critical preload is half the expert's weights; out-blocks 8-15 of stripe 0
run as a third block against the still-resident x.  Head DMAs are few and
large (2-4KB per-partition lines) because walrus shares completion
semaphores across queues -- many small head DMAs serialize on sem reuse.
6 zero-matmul warmups flip the PE HAM clock-gate while the preload
streams.  The final block's last out-blocks drain per-stripe with small
sync-ring stores to shorten the tail.
"""

import numpy as np
import ml_dtypes

import concourse.bass as bass
import concourse.mybir as mybir
import concourse.tile as tile
from concourse.bass_utils import run_bass_kernel_spmd

N_CORES = 8
N_TOKENS = 65536
D_IN = 1024
D_OUT = 2048
N_EXPERTS = 16
E_PER_CORE = N_EXPERTS // N_CORES        # 2
TOK_PER_CORE = N_TOKENS // N_CORES       # 8192
TOK_PER_EXPERT = N_TOKENS // N_EXPERTS   # 4096

P = 128
KP = 4                     # DoubleRow k-pairs (256 contraction each)
TS = 512                   # token stripe (matmul moving free dim)
N_STRIPES = TOK_PER_CORE // TS           # 16
OBW = 128                  # out-feature block (psum partition dim)
N_OB = D_OUT // OBW        # 16

S_X = 4.0                  # keeps x (std 1) in e4m3 normal range
S_W = 128.0                # keeps W (std ~0.0054) out of e4m3 subnormals
OUT_SCALE = 1.0 / (S_X * S_W)

N_WARMUP_MM = 8

F32 = mybir.dt.float32
F16 = mybir.dt.float16
F8 = mybir.dt.float8e4
NP_F8 = ml_dtypes.float8_e4m3

DR = mybir.MatmulPerfMode.DoubleRow
SILU = mybir.ActivationFunctionType.Silu


def _split_multi_waits(nc):
    """This container's walrus encodes at most ONE sync-wait per instruction;
    hoist extras onto single-wait NoOps inserted just before, same engine."""
    for fn in nc.m.functions:
        for bb in fn.blocks:
            insts = list(bb.instructions)
            out = []
            dirty = False
            for inst in insts:
                si = inst.sync_info
                waits = list(si.on_wait) if si and si.on_wait else []
                if len(waits) > 1:
                    dirty = True
                    for j, w in enumerate(waits[:-1]):
                        nop = mybir.InstNoOp(
                            name=f"{inst.name}-prewait{j}", ins=[], outs=[]
                        )
                        nop.engine = inst.engine
                        nop.sync_info = mybir.SyncInfo(on_wait=[w], on_update=[])
                        out.append(nop)
                    inst.sync_info = mybir.SyncInfo(
                        on_wait=[waits[-1]],
                        on_update=list(si.on_update) if si.on_update else [],
                    )
                out.append(inst)
            if dirty:
                bb.instructions = out


def build_kernel():
    nc = bass.Bass()
    # xt[s, kp, p, j*TS+t] = S_X * X[s*TS+t, kp*256 + j*128 + p]
    xt = nc.dram_tensor("xt", [N_STRIPES, KP, P, 2 * TS], F8,
                        kind="ExternalInput")
    # w[e, ob, p, kp*256 + j*128 + i] = S_W * W_e[kp*256 + j*128 + p, ob*128+i]
    w = nc.dram_tensor("w", [E_PER_CORE, N_OB, P, KP * 2 * OBW], F8,
                       kind="ExternalInput")
    # bb[e, p, ob] = bias[ob*128 + p] (exact fp32, applied inside ACT)
    bb = nc.dram_tensor("bb", [E_PER_CORE, P, N_OB], F32, kind="ExternalInput")
    # transposed output; host does yt.T
    yt = nc.dram_tensor("yt", [D_OUT, TOK_PER_CORE], F16, kind="ExternalOutput")

    # block schedule: (expert, [stripe ids], ob_lo, ob_hi)
    blocks = [
        (0, [0], 0, 8),         # head: small critical preload
        (0, [1], 0, 16),
        (0, [0], 8, 16),        # finish stripe 0 against resident x
        (0, [2, 3], 0, 16),
        (0, [4, 5], 0, 16),
        (0, [6, 7], 0, 16),
        (1, [8, 9], 0, 16),
        (1, [10, 11], 0, 16),
        (1, [12, 13], 0, 16),
        (1, [14, 15], 0, 16),   # tail pair; last obs drain per-stripe
    ]

    with tile.TileContext(nc) as tc:
        with (
            tc.tile_pool(name="persist", bufs=1) as persist,
            tc.tile_pool(name="xs", bufs=5) as xsp,
            tc.tile_pool(name="outs", bufs=4) as outs,
            tc.tile_pool(name="psum", bufs=3, space="PSUM") as psump,
        ):
            # --- PE warm-up: matmuls over zeroed scratch, no DMA deps.
            # Sized to keep the PE busy from ~8us until the first real
            # matmul's operands land (~11.5us): an idle PE never reaches
            # the HAM 8/8 clock state and the whole first expert would run
            # at 1.2GHz.
            # N=512 warmups accumulate ~4.3us of PE-busy while the preload
            # streams, flipping the HAM to 8/8 just before the real stream
            # starts.
            zs = persist.tile([P, 2, TS], F8, name="warm_src")
            nc.vector.memset(zs[:], 0.0)
            ps_warm = psump.tile([P, 2, TS], F32, tag="ps2", name="ps_warm")
            for i in range(N_WARMUP_MM):
                nc.tensor.matmul(
                    ps_warm[:, 0, :],
                    lhsT=zs[:, :, 0:P],
                    rhs=zs[:],
                    start=True, stop=True,
                    perf_mode=DR,
                    skip_group_check=True,
                )

            # --- persistent weight/bias tiles ---
            # e0: obs 0-7 as 2-ob tiles (2KB lines, fine head granularity),
            # obs 8-15 and all of e1 as 4-ob tiles (4KB lines).
            w8d = {q: persist.tile([P, 2, KP, 2, OBW], F8, name=f"w8d_{q}")
                   for q in range(4)}
            w8q = {}
            for e in range(E_PER_CORE):
                q0 = 2 if e == 0 else 0
                for q in range(q0, 4):
                    w8q[(e, q)] = persist.tile([P, 4, KP, 2, OBW], F8,
                                               name=f"w8q_{e}_{q}")
            b_sb = [persist.tile([P, N_OB], F32, name=f"bias_{e}")
                    for e in range(E_PER_CORE)]

            def w_ap(e, ob, kp):
                if e == 0 and ob < 8:
                    return w8d[ob // 2][:, ob % 2, kp, :, :]
                return w8q[(e, ob // 4)][:, ob % 4, kp, :, :]

            def load_w2(q, eng):
                eng.dma_start(
                    w8d[q][:],
                    w[0, 2 * q:2 * q + 2].rearrange(
                        "o p (k j i) -> p o k j i", k=KP, j=2))

            def load_w4(e, q, eng):
                eng.dma_start(
                    w8q[(e, q)][:],
                    w[e, 4 * q:4 * q + 4].rearrange(
                        "o p (k j i) -> p o k j i", k=KP, j=2))

            # x tiles: stripes 0/1 as half-stripe (2 k-pair) tiles for head
            # granularity; the rest as full-stripe tiles (4KB lines).
            xh = {}
            x_tiles = {}

            def load_xhalf(s, h, eng=None):
                xh[(s, h)] = xsp.tile([P, 2, 2, TS], F8, tag="xh",
                                      name=f"xh_{s}_{h}")
                (eng or nc.sync).dma_start(
                    xh[(s, h)][:],
                    xt[s, 2 * h:2 * h + 2].rearrange(
                        "k p (j t) -> p k j t", j=2))

            def load_stripe(s):
                x_tiles[s] = xsp.tile([P, KP, 2, TS], F8, tag="xs",
                                      name=f"xs_{s}")
                nc.sync.dma_start(
                    x_tiles[s][:],
                    xt[s].rearrange("k p (j t) -> p k j t", j=2))

            def x_ap(s, kp):
                if s in (0, 1):
                    return xh[(s, kp // 2)][:, kp % 2, :, :]
                return x_tiles[s][:, kp, :, :]

            # --- head preload, need-ordered, few+large DMAs ---
            # The first-matmul operands land in parallel: w obs0-3 (4KB
            # lines) leads the sync queue while x stripe-0 halves lead the
            # scalar queue.  Biases go on the gpsimd software queue so
            # their completion semaphores don't serialize later hardware-
            # queue loads (walrus rotates a small global semaphore pool).
            load_xhalf(0, 0)             # sync: stripe 0, kp 0-1
            load_w2(0, nc.scalar)        # scalar: obs 0-1
            load_xhalf(0, 1)             # sync: stripe 0, kp 2-3
            load_w2(1, nc.sync)          # obs 2-3
            load_w2(2, nc.scalar)        # obs 4-5
            load_w2(3, nc.scalar)        # obs 6-7
            nc.gpsimd.dma_start(b_sb[0][:], bb[0])
            # stripe-1 halves follow on sync; e0 upper weights wait until
            # blkA is underway so they don't contend with its loads
            load_xhalf(1, 0)
            load_xhalf(1, 1)
            nc.gpsimd.dma_start(b_sb[1][:], bb[1])

            # x/w prefetch emitted on sync at the start of block bi
            prefetch = {
                1: [lambda: load_w4(0, 2, nc.sync),
                    lambda: load_w4(0, 3, nc.sync),
                    lambda: load_stripe(2), lambda: load_stripe(3)],
                2: [lambda: load_stripe(4), lambda: load_stripe(5)],
                3: [lambda: load_stripe(6), lambda: load_stripe(7),
                    lambda: load_w4(1, 0, nc.sync),
                    lambda: load_w4(1, 1, nc.sync)],
                4: [lambda: load_stripe(8), lambda: load_stripe(9),
                    lambda: load_w4(1, 2, nc.sync),
                    lambda: load_w4(1, 3, nc.sync)],
                5: [lambda: load_stripe(10), lambda: load_stripe(11)],
                6: [lambda: load_stripe(12), lambda: load_stripe(13)],
                7: [lambda: load_stripe(14), lambda: load_stripe(15)],
            }

            n_blocks = len(blocks)
            for bi, (e, stripes, ob_lo, ob_hi) in enumerate(blocks):
                for fn in prefetch.get(bi, []):
                    fn()
                pair = len(stripes) == 2
                span = len(stripes) * TS
                t0 = stripes[0] * TS
                last_block = bi == n_blocks - 1
                for og in range(ob_lo, ob_hi, 4):
                    obs = list(range(og, min(og + 4, ob_hi)))
                    tail_og = last_block and og + 4 >= ob_hi
                    if not tail_og:
                        tag = "ytp" if pair else "yts"
                        y4 = outs.tile([P, 4, span], F16, tag=tag, name="y4")
                    if pair:
                        for oi, ob in enumerate(obs):
                            ps = psump.tile([P, 2, TS], F32, tag="ps2",
                                            name="ps2")
                            ps_of = [ps[:, 0, :], ps[:, 1, :]]
                            for kp in range(KP):
                                for si in range(2):
                                    nc.tensor.matmul(
                                        ps_of[si],
                                        lhsT=w_ap(e, ob, kp),
                                        rhs=x_ap(stripes[si], kp),
                                        start=(kp == 0),
                                        stop=(kp == KP - 1),
                                        perf_mode=DR,
                                    )
                            bias_ap = b_sb[e][:, ob:ob + 1]
                            if not tail_og:
                                # one ACT drains the whole ob (both banks)
                                nc.scalar.activation(
                                    y4[:, oi, :].rearrange(
                                        "p (s t) -> p s t", s=2),
                                    ps[:], SILU, bias=bias_ap,
                                    scale=OUT_SCALE)
                            elif ob < ob_hi - 2:
                                # tail obs 12-13: per-ob drain + store
                                y1 = outs.tile([P, 2, TS], F16, tag="ytm",
                                               name="y1")
                                nc.scalar.activation(y1[:], ps[:], SILU,
                                                     bias=bias_ap,
                                                     scale=OUT_SCALE)
                                nc.sync.dma_start(
                                    yt[ob * OBW:(ob + 1) * OBW,
                                       t0:t0 + span],
                                    y1[:])
                            else:
                                # final two obs: per-stripe drains + stores
                                # on the now-idle sync ring; the scalar
                                # ring stays ACT-only so the final drain
                                # chain is short
                                for si, s in enumerate(stripes):
                                    ys = outs.tile([P, TS], F16, tag="ytt",
                                                   name="ys")
                                    nc.scalar.activation(
                                        ys[:], ps_of[si], SILU,
                                        bias=bias_ap, scale=OUT_SCALE)
                                    nc.sync.dma_start(
                                        yt[ob * OBW:(ob + 1) * OBW,
                                           s * TS:(s + 1) * TS],
                                        ys[:])
                    else:
                        # stripe block: two obs share one 2-bank psum tile;
                        # each ob's ACT is emitted as soon as its bank's
                        # accumulation group closes
                        s = stripes[0]
                        for half in range(2):
                            ps = psump.tile([P, 2, TS], F32, tag="ps2",
                                            name="ps2")
                            for oi2 in range(2):
                                ob = og + 2 * half + oi2
                                for kp in range(KP):
                                    nc.tensor.matmul(
                                        ps[:, oi2, :],
                                        lhsT=w_ap(e, ob, kp),
                                        rhs=x_ap(s, kp),
                                        start=(kp == 0),
                                        stop=(kp == KP - 1),
                                        perf_mode=DR,
                                    )
                                nc.scalar.activation(
                                    y4[:, 2 * half + oi2, :],
                                    ps[:, oi2, :], SILU,
                                    bias=b_sb[e][:, ob:ob + 1],
                                    scale=OUT_SCALE)
                    if not tail_og:
                        dst = yt[og * OBW:(og + 4) * OBW,
                                 t0:t0 + span].rearrange(
                                     "(o p) t -> p o t", p=P)
                        nc.sync.dma_start(dst, y4[:])

    _split_multi_waits(nc)
    return nc


_NC_CACHE = None


def _get_nc():
    global _NC_CACHE
    if _NC_CACHE is None:
        _NC_CACHE = build_kernel()
    return _NC_CACHE


def _in_maps(sorted_features, routing_matrix, routing_bias):
    maps = []
    for c in range(N_CORES):
        rows = slice(c * TOK_PER_CORE, (c + 1) * TOK_PER_CORE)
        es = slice(c * E_PER_CORE, (c + 1) * E_PER_CORE)
        # [s, kp, p, j*TS+t] = S_X * X[s*TS+t, kp*256 + j*128 + p]
        xt_c = np.ascontiguousarray(
            (sorted_features[rows] * S_X)
            .astype(NP_F8)
            .reshape(N_STRIPES, TS, KP, 2, P)   # [s, t, kp, j, p]
            .transpose(0, 2, 4, 3, 1)           # [s, kp, p, j, t]
            .reshape(N_STRIPES, KP, P, 2 * TS)
        )
        # [e, ob, p, kp*256 + j*128 + i]
        w_c = np.ascontiguousarray(
            (routing_matrix[:, :, es] * S_W)
            .astype(NP_F8)
            .transpose(2, 0, 1)                      # [e, d_in, d_out]
            .reshape(E_PER_CORE, KP, 2, P, N_OB, OBW)  # [e,kp,j,p,ob,i]
            .transpose(0, 4, 3, 1, 2, 5)             # [e, ob, p, kp, j, i]
            .reshape(E_PER_CORE, N_OB, P, KP * 2 * OBW)
        )
        # [e, p, ob] = bias[ob*128 + p]  (exact fp32)
        b_c = np.ascontiguousarray(
            routing_bias[:, es].T                    # [e, d_out]
            .reshape(E_PER_CORE, N_OB, P)
            .transpose(0, 2, 1)
            .astype(np.float32)
        )
        maps.append({"xt": xt_c, "w": w_c, "bb": b_c})
    return maps


def run(sorted_features, routing_matrix, routing_bias, **run_kwargs):
    nc = _get_nc()
    maps = _in_maps(sorted_features, routing_matrix, routing_bias)
    res = run_bass_kernel_spmd(nc, maps, core_ids=list(range(N_CORES)),
                               **run_kwargs)
    out = np.empty((N_TOKENS, D_OUT), dtype=np.float32)
    for c in range(N_CORES):
        yt_c = np.asarray(res.results[c]["yt"])
        out[c * TOK_PER_CORE:(c + 1) * TOK_PER_CORE] = yt_c.T
    return out, res


def kernel(sorted_features, expert_ids_sorted, routing_matrix, routing_bias):
    assert sorted_features.shape == (N_TOKENS, D_IN)
    assert routing_matrix.shape == (D_IN, D_OUT, N_EXPERTS)
    assert routing_bias.shape == (D_OUT, N_EXPERTS)
    out, _ = run(
        np.asarray(sorted_features, dtype=np.float32),
        np.asarray(routing_matrix, dtype=np.float32),
        np.asarray(routing_bias, dtype=np.float32),
    )
    return out
